# revision 9
# baseline (speedup 1.0000x reference)
"""DCT2D kernel v3 for Trainium2 (8 NeuronCores, SPMD data-parallel).

Math: per 8x8 block  out = scale * (C^T (x - 128) C)
  == out_flat[n, uv] = sum_xy (x_flat[n, xy] - 128) * W[xy, uv],
  W[xy, uv] = T[xy, uv] * s[uv].

v3 = v2 with the input quantized to uint8 (host: round(x), exact range
0..255), cutting input HBM traffic to 1 B/elem.  Device dequantizes
u - 128 -> fp16 chunks on the DVE (exact: small ints are fp16-exact,
uint8 input verified supported by probe.py), feeds the fp16 blockdiag
matmul, and converts PSUM fp32 -> int8 (saturating RNE, scale s folded
into W) mostly on ScalarE.  Host multiplies by s on unpack.

Traffic: 6.29 MB in + 6.29 MB out = 12.6 MB/core (~42 us at the ~300
GB/s practical per-core HBM rate).  Input quant adds 3.9e-3 rel err;
total predicted 1.13e-2 vs the 2e-2 gate (simerr.py, bit-exact on the
real data).
"""

import sys

if "/opt/trn_rl_repo" not in sys.path:
    sys.path.insert(0, "/opt/trn_rl_repo")

import numpy as np

import concourse.bass as bass  # noqa: F401
import concourse.mybir as mybir
import concourse.tile as tile
from concourse import bacc
from concourse.bass_utils import run_bass_kernel_spmd

N_CORES = 8
BLOCK = 8
B_DIM = 262144
C_DIM = 3
NBLK = B_DIM * C_DIM          # 786432 total 8x8 blocks
R = NBLK // N_CORES           # 98304 blocks per core
RP = R // 2                   # 49152 packed columns per core
TILE_F = 16384                # columns per SBUF tile
MM_F = 512                    # columns per matmul (one PSUM bank, fp32)
OUT_S = 2.5                   # int8 output scale
# Engine split (probe2-measured: DVE dequant 526 ns, ACT dequant 935 ns,
# ACT psum-convert ~427-500 ns): DVE runs the dequants (its 50.5 us/pass
# is the engine bound), ScalarE runs every psum->int8 convert, plus 1 in
# DQ_ACT_MOD dequants to shave the DVE peak.  0 disables.
DQ_ACT_MOD = 16

_CACHE = {}
last_results = None  # BassKernelResults of the most recent run (for test harness)


def _emit_pass(nc, xpool, qpool, opool, pspool, w_sb, xt, out_t, rp, tile_f):
    """One pass: xt (DRAM u8 [nt,128,tile_f]) -> dequant -> dct -> i8 out."""
    f32 = mybir.dt.float32
    f16 = mybir.dt.float16
    i8 = mybir.dt.int8
    nchunk = tile_f // MM_F
    for t in range(rp // tile_f):
        in_eng, out_eng = (
            (nc.sync, nc.scalar) if t % 2 == 0 else (nc.scalar, nc.sync)
        )
        xin = xpool.tile([128, tile_f], mybir.dt.uint8)
        in_eng.dma_start(xin[:], xt[t])
        osb = opool.tile([128, tile_f], i8)
        for j in range(nchunk):
            sl = slice(j * MM_F, (j + 1) * MM_F)
            xq = qpool.tile([128, MM_F], f16)
            if DQ_ACT_MOD and j % DQ_ACT_MOD == DQ_ACT_MOD - 1:
                nc.scalar.activation(
                    xq[:], xin[:, sl],
                    mybir.ActivationFunctionType.Copy, bias=-128.0,
                )
            else:
                nc.vector.tensor_scalar(
                    xq[:], xin[:, sl], 128.0, None, mybir.AluOpType.subtract
                )
            ps = pspool.tile([128, MM_F], f32)
            nc.tensor.matmul(ps[:], w_sb[:], xq[:], start=True, stop=True)
            nc.scalar.activation(
                osb[:, sl], ps[:], mybir.ActivationFunctionType.Copy
            )
        out_eng.dma_start(out_t[t], osb[:])


def _build_nc(rp=RP, tile_f=TILE_F, n_passes=1, loop_trips=1):
    f16 = mybir.dt.float16
    u8 = mybir.dt.uint8
    i8 = mybir.dt.int8
    nt = rp // tile_f
    nc = bacc.Bacc(None, target_bir_lowering=False, debug=False)
    xt = nc.declare_dram_parameter("xt", [nt, 128, tile_f], u8, isOutput=False)
    w = nc.declare_dram_parameter("w", [128, 128], f16, isOutput=False)
    out = nc.declare_dram_parameter("out", [nt, 128, tile_f], i8, isOutput=True)

    with tile.TileContext(nc) as tc:
        with (
            tc.tile_pool(name="consts", bufs=1) as cpool,
            tc.tile_pool(name="xin", bufs=4) as xpool,
            tc.tile_pool(name="xq", bufs=8) as qpool,
            tc.tile_pool(name="osb", bufs=3) as opool,
            tc.tile_pool(name="ps", bufs=8, space="PSUM") as pspool,
        ):
            w_sb = cpool.tile([128, 128], f16)
            nc.sync.dma_start(w_sb[:], w[:])

            def body():
                for _ in range(n_passes):
                    _emit_pass(
                        nc, xpool, qpool, opool, pspool, w_sb, xt, out, rp, tile_f
                    )

            if loop_trips > 1:
                with tc.For_i(0, loop_trips):
                    body()
            else:
                body()
    nc.compile()
    return nc


def _consts(dct_tensor, scale):
    t_flat = np.asarray(dct_tensor, dtype=np.float64).reshape(64, 64)
    s_flat = np.asarray(scale, dtype=np.float64).reshape(64)
    w64 = (t_flat * s_flat[None, :]) / OUT_S
    w = np.zeros((128, 128), dtype=np.float16)
    w[:64, :64] = w64.astype(np.float16)
    w[64:, 64:] = w64.astype(np.float16)
    return w


def bench_in_maps(seed=0):
    """Representative per-core in_maps (random data) for bench2 timing."""
    rng = np.random.default_rng(seed)
    nt = RP // TILE_F
    xt = rng.integers(0, 256, (nt, 128, TILE_F), dtype=np.uint8)
    w = (rng.standard_normal((128, 128)) * 0.05).astype(np.float16)
    return [{"xt": xt, "w": w} for _ in range(N_CORES)]


def kernel(x, dct_tensor, scale):
    w = _consts(dct_tensor, scale)

    from concurrent.futures import ThreadPoolExecutor

    nt = RP // TILE_F
    xf = np.asarray(x, dtype=np.float32).reshape(NBLK, 64)

    def _pack(c):
        shard8 = np.round(xf[c * R : (c + 1) * R]).astype(np.uint8)
        # xt[t, p*64+k, f] = shard8[2*(t*TILE_F+f)+p, k]
        return np.ascontiguousarray(
            shard8.reshape(nt, TILE_F, 2, 64).transpose(0, 2, 3, 1)
        ).reshape(nt, 128, TILE_F)

    with ThreadPoolExecutor(N_CORES) as pool:
        packs = list(pool.map(_pack, range(N_CORES)))
    in_maps = [{"xt": p, "w": w} for p in packs]

    if "nc" not in _CACHE:
        _CACHE["nc"] = _build_nc()
    res = run_bass_kernel_spmd(_CACHE["nc"], in_maps, core_ids=list(range(N_CORES)))
    global last_results
    last_results = res

    full = np.empty((NBLK, 64), dtype=np.float32)

    def _unpack(c):
        o = np.asarray(res.results[c]["out"])  # [nt, 128, TILE_F] int8 packed
        full[c * R : (c + 1) * R] = (
            o.reshape(nt, 2, 64, TILE_F).transpose(0, 3, 1, 2).reshape(R, 64)
        ).astype(np.float32) * np.float32(OUT_S)

    with ThreadPoolExecutor(N_CORES) as pool:
        list(pool.map(_unpack, range(N_CORES)))
    return full.reshape(B_DIM, C_DIM, BLOCK, BLOCK)


# revision 11
# speedup vs baseline: 1.2240x; 1.2240x over previous
"""DCT2D kernel v2 for Trainium2 (8 NeuronCores, SPMD data-parallel).

Math: per 8x8 block  out = scale * (C^T (x - 128) C)
  == flat form:  out_flat[n, uv] = sum_xy (x_flat[n, xy] - 128) * W[xy, uv]
  with W[xy, uv] = T[xy, uv] * s[uv].

v2 cuts HBM traffic (the v1 bottleneck: 50.3 MB/core fp32 I/O ~ 141 us
floor, measured 161-164 us) by quantizing the device I/O:
  - input:  host precomputes o = x - 128 in fp16 (quant err ~2.4e-4 rel)
    and packs two blocks per column -> [nt, 128, TILE_F] fp16, 2 B/elem.
  - weights: blockdiag(W/s, W/s) in fp16 -> PE runs at 1 cycle/row
    (4x faster than fp32's 4 cycles/row, PE ~35 us/pass, off the
    critical path).
  - output: PSUM fp32 -> int8 with scale s folded into W.  HW float->int8
    conversion is saturating RNE on both DVE and ScalarE (verified on HW
    by probe.py), so plain converts alternate between the two engines
    (PSUM has one DVE read port -> each engine ~26 us/pass).  1 B/elem.
    Host multiplies by s on unpack.
Total 18.9 MB/core -> ~53 us HBM floor.  Rel err ~1.06e-2 vs the 2e-2
gate (quantization chain simulated bit-exactly on the real data in
simerr.py; s=2.5 clips 5810 of 50.3M outputs, saturation handles them).
"""

import sys

if "/opt/trn_rl_repo" not in sys.path:
    sys.path.insert(0, "/opt/trn_rl_repo")

import numpy as np

import concourse.bass as bass  # noqa: F401
import concourse.mybir as mybir
import concourse.tile as tile
from concourse import bacc
from concourse.bass_utils import run_bass_kernel_spmd

N_CORES = 8
BLOCK = 8
B_DIM = 262144
C_DIM = 3
NBLK = B_DIM * C_DIM          # 786432 total 8x8 blocks
R = NBLK // N_CORES           # 98304 blocks per core
RP = R // 2                   # 49152 packed columns per core
TILE_F = 16384                # columns per SBUF tile (4 MiB fp16 in-DMA)
MM_F = 512                    # columns per matmul (one PSUM bank, fp32)
PS_W = 1024                   # columns per PSUM->int8 convert op (2 banks);
                              # probe3: wide converts amortize per-op cost
                              # (ACT 789 -> 574 ns per 512 cols)
OUT_S = 2.5                   # int8 output scale

_CACHE = {}
last_results = None  # BassKernelResults of the most recent run (for test harness)


def _emit_pass(nc, xpool, opool, pspool, w_sb, xt, out_t, rp, tile_f):
    """One full pass: xt (DRAM fp16 [nt,128,tile_f]) -> dct -> int8 out.

    The two HWDGE rings (sync, scalar) are byte-balanced: alternate tiles
    swap which ring carries the 2-byte input vs the 1-byte output so each
    ring moves ~9.4 MB/pass.
    """
    f32 = mybir.dt.float32
    i8 = mybir.dt.int8
    for t in range(rp // tile_f):
        in_eng, out_eng = (
            (nc.sync, nc.scalar) if t % 2 == 0 else (nc.scalar, nc.sync)
        )
        xin = xpool.tile([128, tile_f], mybir.dt.float16)
        in_eng.dma_start(xin[:], xt[t])
        osb = opool.tile([128, tile_f], i8)
        mm_per_group = PS_W // MM_F
        for g in range(tile_f // PS_W):
            ps = pspool.tile([128, PS_W], f32)
            for k in range(mm_per_group):
                j = g * mm_per_group + k
                nc.tensor.matmul(
                    ps[:, k * MM_F : (k + 1) * MM_F],
                    w_sb[:],
                    xin[:, j * MM_F : (j + 1) * MM_F],
                    start=True, stop=True,
                )
            dst = osb[:, g * PS_W : (g + 1) * PS_W]
            if g % 2 == 0:
                nc.vector.tensor_scalar_mul(dst, ps[:], 1.0)
            else:
                nc.scalar.activation(
                    dst, ps[:], mybir.ActivationFunctionType.Copy
                )
        out_eng.dma_start(out_t[t], osb[:])


def _build_nc(rp=RP, tile_f=TILE_F, n_passes=1, loop_trips=1):
    f16 = mybir.dt.float16
    i8 = mybir.dt.int8
    nt = rp // tile_f
    nc = bacc.Bacc(None, target_bir_lowering=False, debug=False)
    xt = nc.declare_dram_parameter("xt", [nt, 128, tile_f], f16, isOutput=False)
    w = nc.declare_dram_parameter("w", [128, 128], f16, isOutput=False)
    out = nc.declare_dram_parameter("out", [nt, 128, tile_f], i8, isOutput=True)

    with tile.TileContext(nc) as tc:
        with (
            tc.tile_pool(name="consts", bufs=1) as cpool,
            tc.tile_pool(name="xin", bufs=4) as xpool,
            tc.tile_pool(name="osb", bufs=3) as opool,
            tc.tile_pool(name="ps", bufs=8 * MM_F // PS_W, space="PSUM") as pspool,
        ):
            w_sb = cpool.tile([128, 128], f16)
            nc.sync.dma_start(w_sb[:], w[:])

            def body():
                for _ in range(n_passes):
                    _emit_pass(nc, xpool, opool, pspool, w_sb, xt, out, rp, tile_f)

            if loop_trips > 1:
                with tc.For_i(0, loop_trips):
                    body()
            else:
                body()
    nc.compile()
    return nc


def _consts(dct_tensor, scale):
    t_flat = np.asarray(dct_tensor, dtype=np.float64).reshape(64, 64)
    s_flat = np.asarray(scale, dtype=np.float64).reshape(64)
    w64 = (t_flat * s_flat[None, :]) / OUT_S
    w = np.zeros((128, 128), dtype=np.float16)
    w[:64, :64] = w64.astype(np.float16)
    w[64:, 64:] = w64.astype(np.float16)
    return w


def bench_in_maps(seed=0):
    """Representative per-core in_maps (random data) for bench2 timing."""
    rng = np.random.default_rng(seed)
    nt = RP // TILE_F
    xt = ((rng.random((nt, 128, TILE_F), dtype=np.float32) * 255.0) - 128.0).astype(
        np.float16
    )
    w = (rng.standard_normal((128, 128)) * 0.05).astype(np.float16)
    return [{"xt": xt, "w": w} for _ in range(N_CORES)]


def kernel(x, dct_tensor, scale):
    w = _consts(dct_tensor, scale)

    from concurrent.futures import ThreadPoolExecutor

    nt = RP // TILE_F
    xf = np.asarray(x, dtype=np.float32).reshape(NBLK, 64)

    def _pack(c):
        shard16 = (xf[c * R : (c + 1) * R] - 128.0).astype(np.float16)
        # xt[t, p*64+k, f] = shard16[2*(t*TILE_F+f)+p, k]
        return np.ascontiguousarray(
            shard16.reshape(nt, TILE_F, 2, 64).transpose(0, 2, 3, 1)
        ).reshape(nt, 128, TILE_F)

    with ThreadPoolExecutor(N_CORES) as pool:
        packs = list(pool.map(_pack, range(N_CORES)))
    in_maps = [{"xt": p, "w": w} for p in packs]

    if "nc" not in _CACHE:
        _CACHE["nc"] = _build_nc()
    res = run_bass_kernel_spmd(_CACHE["nc"], in_maps, core_ids=list(range(N_CORES)))
    global last_results
    last_results = res

    full = np.empty((NBLK, 64), dtype=np.float32)

    def _unpack(c):
        o = np.asarray(res.results[c]["out"])  # [nt, 128, TILE_F] int8 packed
        full[c * R : (c + 1) * R] = (
            o.reshape(nt, 2, 64, TILE_F).transpose(0, 3, 1, 2).reshape(R, 64)
        ).astype(np.float32) * np.float32(OUT_S)

    with ThreadPoolExecutor(N_CORES) as pool:
        list(pool.map(_unpack, range(N_CORES)))
    return full.reshape(B_DIM, C_DIM, BLOCK, BLOCK)
